# revision 24
# baseline (speedup 1.0000x reference)
"""Bass/Trainium2 kernel for the FDE "fractal noprop" dense-MLP network.

Strategy: data-parallel over the batch dim across 8 NeuronCores (256
rows/core), weights replicated.  Everything on-device is kept
feature-major ([128 partitions, feat_chunk, batch]) so activations come
out of each GEMM already in the layout the next GEMM consumes - no
on-device transposes.  Matmul operands are fp16 (fp32 PSUM accumulate,
fp32 z accumulator), which measures ~4e-4 max relative error end to end.

Host-side preprocessing (untimed): shard the batch, cast/pack weights
into per-(m,k) 128x128 SBUF tile layout, fold alpha_t into wB/bB, fold
sqrt(1-alpha_t) into the noise.
"""

import os
import sys
from contextlib import ExitStack

import numpy as np

try:
    import concourse.bass as bass
except ImportError:  # pragma: no cover - fresh-dir fallback
    sys.path.append("/opt/trn_rl_repo")
    import concourse.bass as bass

import concourse.tile as tile
from concourse import bacc, mybir
from concourse.bass_utils import run_bass_kernel_spmd

P = 128
F32 = mybir.dt.float32
F16 = mybir.dt.float16
ACT = mybir.ActivationFunctionType
ALU = mybir.AluOpType

# Full problem dims (hardcoded per harness contract).
B, IN_DIM, H, OUT_DIM, T = 2048, 1024, 2048, 1024, 10
NCORES = 8


def _alphas(t_steps):
    return np.linspace(0.99, 0.9, t_steps).astype(np.float32)


# ---------------------------------------------------------------------------
# Bass program
# ---------------------------------------------------------------------------

def build_bass(bc, in_dim, h, out_dim, t_steps):
    """Build the single-core SPMD program. All dims multiples of 128."""
    nc = bacc.Bacc("TRN2", target_bir_lowering=False, debug=False)
    KI, KH, KO = in_dim // P, h // P, out_dim // P
    alpha = _alphas(t_steps)

    def din(name, shape, dt):
        return nc.dram_tensor(name, shape, dt, kind="ExternalInput").ap()

    xT = din("xT", [P, KI, bc], F16)
    z0T = din("z0T", [P, KH, bc], F32)
    nzT = din("nzT", [t_steps, P, KH, bc], F32)
    w1 = din("w1", [KH, P, KI, P], F16)
    w2 = din("w2", [KH, P, KH, P], F16)
    wAz = din("wAz", [t_steps, KH, P, KH, P], F16)
    wAx = din("wAx", [t_steps, KH, P, KH, P], F16)
    wBs = din("wBs", [t_steps, KH, P, KH, P], F16)
    wC = din("wC", [KO, P, KH, P], F16)
    b1 = din("b1", [P, KH], F32)
    b2 = din("b2", [P, KH], F32)
    bA = din("bA", [P, t_steps, KH], F32)
    bBs = din("bBs", [P, t_steps, KH], F32)
    bC = din("bC", [P, KO], F32)
    outT = nc.dram_tensor("outT", [P, KO, bc], F32, kind="ExternalOutput").ap()

    with tile.TileContext(nc) as tc, ExitStack() as ctx:
        const = ctx.enter_context(tc.tile_pool(name="const", bufs=1))
        state = ctx.enter_context(tc.tile_pool(name="state", bufs=1))
        wpool = ctx.enter_context(tc.tile_pool(name="wpool", bufs=10))
        npool = ctx.enter_context(tc.tile_pool(name="npool", bufs=2))
        upool = ctx.enter_context(tc.tile_pool(name="upool", bufs=2))
        psum = ctx.enter_context(tc.tile_pool(name="psum", bufs=8, space="PSUM"))

        # Persistent state (feature-major)
        z = state.tile([P, KH, bc], F32)
        zh = state.tile([P, KH, bc], F16)
        xe = state.tile([P, KH, bc], F16)
        hb = state.tile([P, KH, bc], F16)
        xt = state.tile([P, KI, bc], F16)
        ob = state.tile([P, KO, bc], F32)
        b1s = const.tile([P, KH], F32)
        b2s = const.tile([P, KH], F32)
        bCs = const.tile([P, KO], F32)
        # all per-block biases loaded once up front: per-block bias DMAs
        # would add a third sem wait to their consumers (HW limit is 2)
        bAall = const.tile([P, t_steps, KH], F32)
        bBall = const.tile([P, t_steps, KH], F32)

        nc.sync.dma_start(xt[:], xT)
        nc.sync.dma_start(z[:], z0T)
        nc.sync.dma_start(b1s[:], b1)
        nc.sync.dma_start(b2s[:], b2)
        nc.sync.dma_start(bCs[:], bC)
        nc.sync.dma_start(bAall[:], bA)
        nc.sync.dma_start(bBall[:], bBs)
        nc.vector.tensor_copy(zh[:], z[:])
        # Touch the block-bias tables from ACT/DVE once, right after their
        # load: advances those engines' clocks past the DMA so the hot-loop
        # consumers don't each need a 3rd sem wait (HW limit is 2/inst).
        scratch = const.tile([P, 2], F32)
        nc.scalar.activation(scratch[:, 0:1], bAall[:, 0, 0:1], ACT.Identity)
        nc.vector.tensor_copy(scratch[:, 1:2], bBall[:, 0, 0:1])

        # CoreSim has no Silu table; KERNEL_SIM_SILU=1 swaps in an
        # equivalent sigmoid+multiply pair for simulator runs.
        sim_silu = bool(int(os.environ.get("KERNEL_SIM_SILU", "0")))

        def emit_silu(dst, pt, bias_ap):
            """dst = silu(mm + bias), mm in the first half of a full-bank
            psum tile (the second half is scratch for the sim fallback)."""
            mm = pt[:, :bc]
            if sim_silu:
                s = pt[:, bc : 2 * bc]
                nc.scalar.activation(s, mm, ACT.Sigmoid, bias=bias_ap)
                nc.vector.scalar_tensor_tensor(dst, mm, bias_ap, s, ALU.add, ALU.mult)
            else:
                nc.scalar.activation(dst, mm, ACT.Silu, bias=bias_ap)

        def gemm_tile(wdram_slice, rhs, nk, pt=None, start=True, stop=True,
                      pool=None, tag="w"):
            """One 128-row output tile: accumulate nk K-chunks into psum."""
            wt = (pool or wpool).tile([P, nk, P], F16, tag=tag)
            nc.sync.dma_start(wt[:], wdram_slice)
            if pt is None:
                pt = psum.tile([P, 2 * bc], F32, tag="pt")
            for s in range(nk):
                nc.tensor.matmul(
                    pt[:, :bc], wt[:, s, :], rhs[:, s, :],
                    start=(start and s == 0), stop=(stop and s == nk - 1),
                )
            return pt

        # --- input embed: hb = silu(x @ w1 + b1); xe = hb @ w2 + b2
        for m in range(KH):
            pt = gemm_tile(w1[m], xt, KI)
            emit_silu(hb[:, m, :], pt, b1s[:, m : m + 1])
        for m in range(KH):
            pt = gemm_tile(w2[m], hb, KH)
            nc.scalar.activation(
                xe[:, m, :], pt[:, :bc], ACT.Identity, bias=b2s[:, m : m + 1]
            )

        # --- T noprop blocks
        for t in range(t_steps):
            nt = npool.tile([P, KH, bc], F32, tag="nz")
            nc.sync.dma_start(nt[:], nzT[t])
            u = upool.tile([P, KH, bc], F16, tag="u")

            # GEMM1: psum[m] = wAx[t,m].T @ xe + wAz[t,m].T @ zh, then
            # u[m] = silu(psum[m] + bA).  The x half has no dependency on
            # this block's z, so emit it one tile ahead: the PE crosses the
            # inter-block z dependency without going idle.
            pts = {}

            def emit_x(m, t=t):
                pts[m] = gemm_tile(wAx[t, m], xe, KH, start=True, stop=False)

            def emit_z(m, t=t, u=u):
                gemm_tile(wAz[t, m], zh, KH, pt=pts[m], start=False, stop=True)
                emit_silu(u[:, m, :], pts.pop(m), bAall[:, t, m : m + 1])

            emit_x(0)
            for m in range(KH):
                if m + 1 < KH:
                    emit_x(m + 1)
                emit_z(m)

            # z <- (1-a_t) * z + noise_scaled[t]   (DVE, runs under GEMM2)
            za = float(1.0 - alpha[t])
            for m in range(KH):
                nc.vector.scalar_tensor_tensor(
                    z[:, m, :], z[:, m, :], za, nt[:, m, :], ALU.mult, ALU.add
                )

            # GEMM2 (wB pre-scaled by a_t): z += psum + a_t*bB; zh = fp16(z)
            for mo in range(KH):
                pt = gemm_tile(wBs[t, mo], u, KH)
                nc.vector.scalar_tensor_tensor(
                    z[:, mo, :], pt[:, :bc], bBall[:, t, mo : mo + 1], z[:, mo, :],
                    ALU.add, ALU.add,
                )
                nc.vector.tensor_copy(zh[:, mo, :], z[:, mo, :])

        # --- classifier
        for m in range(KO):
            pt = gemm_tile(wC[m], zh, KH)
            nc.scalar.activation(
                ob[:, m, :], pt[:, :bc], ACT.Identity, bias=bCs[:, m : m + 1]
            )
        nc.sync.dma_start(outT, ob[:])

    nc.compile()
    return nc


# ---------------------------------------------------------------------------
# Host-side packing
# ---------------------------------------------------------------------------

def _pack_w(w, dtype=np.float16):
    """[K, M] -> [M//P, P, K//P, P] tile layout: [m][p, s, j] = w[s*P+p, m*P+j]."""
    K, M = w.shape
    return np.ascontiguousarray(
        w.astype(dtype).reshape(K // P, P, M // P, P).transpose(2, 1, 0, 3)
    )


def _pack_wT(w, dtype=np.float16):
    """[T, K, M] -> [T, M//P, P, K//P, P]."""
    t, K, M = w.shape
    return np.ascontiguousarray(
        w.astype(dtype).reshape(t, K // P, P, M // P, P).transpose(0, 3, 2, 1, 4)
    )


def _pack_actT(a, dtype):
    """[Bc, F] -> [P, F//P, Bc]: [p, k, b] = a[b, k*P+p]."""
    Bc, F = a.shape
    return np.ascontiguousarray(
        a.astype(dtype).T.reshape(F // P, P, Bc).transpose(1, 0, 2)
    )


def _pack_bias(b):
    """[F] -> [P, F//P]."""
    return np.ascontiguousarray(b.astype(np.float32).reshape(-1, P).T)


def make_inputs(inputs, n_cores, t_steps):
    """Returns (shared dict, list of per-core dicts)."""
    alpha = _alphas(t_steps)
    ns = np.sqrt(1.0 - alpha).astype(np.float32)

    wA = np.asarray(inputs["wA"], np.float32)
    h = wA.shape[2]
    shared = {
        "w1": _pack_w(np.asarray(inputs["w1_in"], np.float32)),
        "w2": _pack_w(np.asarray(inputs["w2_in"], np.float32)),
        "wAz": _pack_wT(wA[:, :h, :]),
        "wAx": _pack_wT(wA[:, h:, :]),
        "wBs": _pack_wT(alpha[:, None, None] * np.asarray(inputs["wB"], np.float32)),
        "wC": _pack_w(np.asarray(inputs["wC"], np.float32)),
        "b1": _pack_bias(np.asarray(inputs["b1_in"])),
        "b2": _pack_bias(np.asarray(inputs["b2_in"])),
        "bA": np.ascontiguousarray(
            np.stack([_pack_bias(b) for b in np.asarray(inputs["bA"], np.float32)])
            .transpose(1, 0, 2)
        ),
        "bBs": np.ascontiguousarray(
            np.stack(
                [_pack_bias(alpha[i] * np.asarray(inputs["bB"], np.float32)[i])
                 for i in range(t_steps)]
            ).transpose(1, 0, 2)
        ),
        "bC": _pack_bias(np.asarray(inputs["bC"])),
    }

    x = np.asarray(inputs["x"], np.float32)
    z0 = np.asarray(inputs["z0"], np.float32)
    noise = np.asarray(inputs["noise"], np.float32)
    b_total = x.shape[0]
    bc = b_total // n_cores
    kh = z0.shape[1] // P

    in_maps = []
    for c in range(n_cores):
        bs = slice(c * bc, (c + 1) * bc)
        nz = noise[:, bs, :] * ns[:, None, None]  # [T, bc, H] fp32
        nz = np.ascontiguousarray(
            nz.transpose(0, 2, 1).reshape(t_steps, kh, P, bc).transpose(0, 2, 1, 3)
        )
        m = dict(shared)
        m["xT"] = _pack_actT(x[bs], np.float16)
        m["z0T"] = _pack_actT(z0[bs], np.float32)
        m["nzT"] = nz
        in_maps.append(m)
    return in_maps


def unpack_output(results, out_dim, n_cores):
    outs = []
    for c in range(n_cores):
        o = results[c]["outT"]  # [P, KO, bc]
        outs.append(o.transpose(1, 0, 2).reshape(out_dim, -1).T)  # [bc, OUT]
    return np.ascontiguousarray(np.concatenate(outs, axis=0), dtype=np.float32)


# ---------------------------------------------------------------------------
# Entry point
# ---------------------------------------------------------------------------

_NC_CACHE = {}


def _get_nc():
    key = (B // NCORES, IN_DIM, H, OUT_DIM, T)
    if key not in _NC_CACHE:
        _NC_CACHE[key] = build_bass(*key)
    return _NC_CACHE[key]


def kernel(**inputs):
    nc = _get_nc()
    in_maps = make_inputs(inputs, NCORES, T)
    trace = bool(int(os.environ.get("KERNEL_TRACE", "0")))
    tmpdir = os.environ.get("KERNEL_TRACE_DIR") or None
    res = run_bass_kernel_spmd(
        nc, in_maps, core_ids=list(range(NCORES)), trace=trace, tmpdir=tmpdir
    )
    if trace:
        kernel.last_results = res
    return unpack_output(res.results, OUT_DIM, NCORES)
